# revision 7
# baseline (speedup 1.0000x reference)
"""Trainium2 Bass kernel for CommunicativeMessagePassing (D-MPNN bond-message GNN).

Self-contained: takes full inputs, shards across 8 NeuronCores, returns full output.

Math (dead code removed -- the reference's H_a / a_max / gate chain never reaches
the output):
    H_b = relu(concat(V[v], E_feat) @ Wi_bond.T)
    2x:  a_sum = segment_sum(H_b, w); H_b += relu((H_b + (a_sum[v] - H_b[rev]) @ Wh.T) @ Wf.T)
    a_sum = segment_sum(H_b, w); out = relu(concat(V, a_sum) @ Wo_atom.T)

v3 (T-exchange): every edge update is computed exactly ONCE, on the core owning
its destination atom. Per-edge update relu(Wf.h_e + A4[v_e] - C.h_rev(e)) with
C = Wf@Wh needs A4[v_e] - C.h_rev(e), both local to the core owning rev(e)
(dst of rev(e) is v_e). That core computes T_rev = A4[dst] - C.h for each of its
edges and ships the 512B row to the home core of the paired edge through
dma_scatter_add -> AllToAll -> dma_gather(transpose=True). The transposing
gather lands T directly feature-major, so the receive side is a plain PSUM add.
No mirrored R-state, no R-half init, no A4 AllGather, no transpose matmuls on
the receive path.

Sharding: atoms split into 8 contiguous ranges; a core owns the edges whose dst
atom is local, columns in round-major degree-sorted order for dense segment-sum.
The AllToAll is split in two (by sender column half) so the first half flies
while the second half's T is still being computed.
"""
import sys
sys.path.insert(0, "/opt/trn_rl_repo")
import numpy as np
import ml_dtypes
BF = ml_dtypes.bfloat16

NCORES = 8
P = 128
CHUNK = 512
DH = 256

_cache = {}


def _wrap_idx(idx):
    """Pack an index list into the [128, n/16] wrapped+replicated int16 layout."""
    a = np.asarray(idx, np.int16).reshape(-1, 16).T          # [16, n/16]
    return np.tile(a, (8, 1)).copy()


# ---------------------------------------------------------------- host preprocessing
def _preprocess(V, E_feat, edge_index, rev_edge_index):
    N, DV = V.shape
    E, DE = E_feat.shape
    v = np.asarray(edge_index[0], np.int64)
    w = np.asarray(edge_index[1], np.int64)
    rev = np.asarray(rev_edge_index, np.int64)
    ASH = N // NCORES
    APAD = ((ASH + P - 1) // P) * P

    shard = w // ASH

    per = []
    for c in range(NCORES):
        eids = np.nonzero(shard == c)[0]
        wl = w[eids] - c * ASH
        deg = np.bincount(wl, minlength=ASH)
        order = np.argsort(-deg, kind="stable")      # rank -> atom (local)
        rank_of = np.empty(ASH, np.int64)
        rank_of[order] = np.arange(ASH)
        ar = rank_of[wl]
        o2 = np.lexsort((eids, ar))
        eids_s, ar_s = eids[o2], ar[o2]
        if len(ar_s):
            runs = np.r_[0, np.nonzero(np.diff(ar_s))[0] + 1]
            lens = np.diff(np.r_[runs, len(ar_s)])
            pos = np.arange(len(ar_s)) - np.repeat(runs, lens)
        else:
            pos = np.zeros(0, np.int64)
        per.append(dict(eids=eids_s, ar=ar_s, pos=pos, deg=deg, order=order))

    maxdeg = max(int(p["deg"].max()) for p in per) if E else 0
    n_r = np.zeros(maxdeg, np.int64)
    for p in per:
        cnt = np.bincount(p["deg"], minlength=maxdeg + 1)
        gt = ASH - np.cumsum(cnt)[:maxdeg]
        n_r = np.maximum(n_r, gt)
    starts = np.r_[0, np.cumsum(n_r)]
    K = int(starts[-1])
    KP = ((K + CHUNK - 1) // CHUNK) * CHUNK
    NT = KP // CHUNK
    HALF = (NT // 2) * CHUNK

    # per-core column tables
    for c in range(NCORES):
        p = per[c]
        cols = starts[p["pos"]] + p["ar"]
        L_eid = np.full(KP, -1, np.int64)
        L_eid[cols] = p["eids"]
        p["L_eid"] = L_eid
        p["dd"] = np.where(L_eid >= 0, v[np.maximum(L_eid, 0)] // ASH, -1)

    # exchange block sizes: max rows for any (sender, dest) pair per column half
    maxA = maxB = 0
    for c in range(NCORES):
        dd, Le = per[c]["dd"], per[c]["L_eid"]
        for d in range(NCORES):
            mA = (dd == d) & (np.arange(KP) < HALF)
            mB = (dd == d) & (np.arange(KP) >= HALF)
            maxA = max(maxA, int(mA.sum()))
            maxB = max(maxB, int(mB.sum()))
    BA, BB = maxA + 2, maxB + 2          # +trash slot, +guaranteed-zero slot
    NR = NCORES * (BA + BB)
    assert NR < 32768, NR
    ZERO_ROW = BA - 2                    # sender-0 region-A zero slot

    # sender slot assignment + global row of each edge's T value
    rowtab = np.full(E, -1, np.int64)
    for c in range(NCORES):
        dd, Le = per[c]["dd"], per[c]["L_eid"]
        SC = np.empty(KP, np.int64)
        colr = np.arange(KP)
        for base, Breg, m_reg, gbase in ((0, BA, colr < HALF, 0),
                                         (0, BB, colr >= HALF, NCORES * BA)):
            SC[m_reg] = Breg - 1         # pads -> region trash slot (block 0)
            for d in range(NCORES):
                m = (dd == d) & m_reg
                idxs = np.nonzero(m)[0]
                SC[idxs] = d * Breg + np.arange(len(idxs))
                # receiver-side row: AllToAll puts sender c's block at c*Breg
                rowtab[Le[idxs]] = gbase + c * Breg + np.arange(len(idxs))
        per[c]["SC"] = SC

    cores = []
    for c in range(NCORES):
        p = per[c]
        L_eid = p["L_eid"]
        mask = L_eid >= 0
        GI = np.full(KP, ZERO_ROW, np.int64)
        GI[mask] = rowtab[rev[L_eid[mask]]]
        assert GI.min() >= 0 and GI.max() < NR

        # X staging, feature-major [DV+DE, KP]
        X = np.zeros((DV + DE, KP), BF)
        le = L_eid[mask]
        X[:DV][:, mask] = V[v[le]].T.astype(BF)
        X[DV:][:, mask] = E_feat[le].T.astype(BF)

        Vfm = np.zeros((DV, APAD), BF)
        Vfm[:, :ASH] = V[c * ASH + p["order"]].T.astype(BF)
        cores.append(dict(scidx=_wrap_idx(p["SC"]), giidx=_wrap_idx(GI),
                          X=X, Vfm=Vfm, order=p["order"]))

    return dict(N=N, E=E, DV=DV, DE=DE, ASH=ASH, APAD=APAD, KP=KP,
                BA=BA, BB=BB, NR=NR,
                starts=starts.astype(np.int64), n_r=n_r, cores=cores)


def _weights(Wi_bond, Wh_bond, Wf_bond, Wo_atom, DV):
    C = (Wf_bond @ Wh_bond).astype(np.float32)
    return dict(
        WiT=np.ascontiguousarray(Wi_bond.T.astype(BF)),       # [DV+DE, DH]
        WfT=np.ascontiguousarray(Wf_bond.T.astype(BF)),       # [DH, DH]
        CT=np.ascontiguousarray(C.T.astype(BF)),
        CnT=np.ascontiguousarray((-C.T).astype(BF)),
        WoTv=np.ascontiguousarray(Wo_atom.T[:DV].astype(BF)),  # [DV, DH]
        WoTs=np.ascontiguousarray(Wo_atom.T[DV:].astype(BF)),  # [DH, DH]
        identb=np.eye(P, dtype=BF),
    )


# ---------------------------------------------------------------- bass program
def _build(meta, DEPTH_ITERS=2):
    import concourse.bass as bass
    import concourse.bacc as bacc
    import concourse.tile as tile
    from concourse import mybir

    F32 = mybir.dt.float32
    BF16 = mybir.dt.bfloat16
    I16 = mybir.dt.int16
    KP, APAD, DV, DE = meta["KP"], meta["APAD"], meta["DV"], meta["DE"]
    BA, BB, NR = meta["BA"], meta["BB"], meta["NR"]
    starts, n_r = meta["starts"], meta["n_r"]
    R_ROUNDS = len(n_r)
    DXT = DV + DE
    NFH = DH // P                      # feature halves (2)
    NA_CH = (APAD + CHUNK - 1) // CHUNK
    NT = KP // CHUNK
    NTA = NT // 2                      # chunks in exchange region A
    NW = CHUNK // 16                   # idx columns per chunk (32)

    nc = bacc.Bacc("TRN2", target_bir_lowering=False, debug=False, num_devices=NCORES)

    x_in = nc.dram_tensor("x", [DXT, KP], BF16, kind="ExternalInput")
    vfm_in = nc.dram_tensor("vfm", [DV, APAD], BF16, kind="ExternalInput")
    scidx_in = nc.dram_tensor("scidx", [P, NT * NW], I16, kind="ExternalInput")
    giidx_in = nc.dram_tensor("giidx", [P, NT * NW], I16, kind="ExternalInput")
    wiT_in = nc.dram_tensor("wiT", [DXT, DH], BF16, kind="ExternalInput")
    wfT_in = nc.dram_tensor("wfT", [DH, DH], BF16, kind="ExternalInput")
    ct_in = nc.dram_tensor("cT", [DH, DH], BF16, kind="ExternalInput")
    cnT_in = nc.dram_tensor("cnT", [DH, DH], BF16, kind="ExternalInput")
    woTv_in = nc.dram_tensor("woTv", [DV, DH], BF16, kind="ExternalInput")
    woTs_in = nc.dram_tensor("woTs", [DH, DH], BF16, kind="ExternalInput")
    idb_in = nc.dram_tensor("identb", [P, P], BF16, kind="ExternalInput")
    out_ext = nc.dram_tensor("out", [DH, APAD], BF16, kind="ExternalOutput")

    voutb = nc.dram_tensor("voutb", [DH, APAD], BF16)
    ccin = [nc.dram_tensor(f"ccin{i}", [NR, DH], BF16) for i in range(DEPTH_ITERS)]
    ccout = [nc.dram_tensor(f"ccout{i}", [NR, DH], BF16) for i in range(DEPTH_ITERS)]

    with tile.TileContext(nc) as tc:
        with (
            tc.tile_pool(name="wpool", bufs=1) as wp,
            tc.tile_pool(name="state", bufs=1) as st,
            tc.tile_pool(name="hin", bufs=3) as hp,
            tc.tile_pool(name="rout", bufs=3) as rp,
            tc.tile_pool(name="gpool", bufs=2) as gp,
            tc.tile_pool(name="psum", bufs=3, space="PSUM") as ps,
        ):
            # ---- resident weights
            wiT0 = wp.tile([P, DH], BF16, name="wiT0")
            nc.sync.dma_start(out=wiT0[:], in_=wiT_in[:P, :])
            wiT1 = wp.tile([DXT - P, DH], BF16, name="wiT1")
            nc.sync.dma_start(out=wiT1[:], in_=wiT_in[P:DXT, :])
            wfT = [wp.tile([P, DH], BF16, tag=f"wf{k}", name=f"wf{k}") for k in range(NFH)]
            cTb = [wp.tile([P, DH], BF16, tag=f"ct{k}", name=f"ct{k}") for k in range(NFH)]
            cnT = [wp.tile([P, DH], BF16, tag=f"cn{k}", name=f"cn{k}") for k in range(NFH)]
            for k in range(NFH):
                nc.sync.dma_start(out=wfT[k][:], in_=wfT_in[k * P:(k + 1) * P, :])
                nc.sync.dma_start(out=cTb[k][:], in_=ct_in[k * P:(k + 1) * P, :])
                nc.sync.dma_start(out=cnT[k][:], in_=cnT_in[k * P:(k + 1) * P, :])
            woTv0 = wp.tile([P, DH], BF16, name="woTv0")
            nc.sync.dma_start(out=woTv0[:], in_=woTv_in[:P, :])
            woTv1 = wp.tile([DV - P, DH], BF16, name="woTv1")
            nc.sync.dma_start(out=woTv1[:], in_=woTv_in[P:DV, :])
            woTs = [wp.tile([P, DH], BF16, tag=f"wo{k}", name=f"wo{k}") for k in range(NFH)]
            for k in range(NFH):
                nc.sync.dma_start(out=woTs[k][:], in_=woTs_in[k * P:(k + 1) * P, :])
            identb = wp.tile([P, P], BF16, name="identb")
            nc.sync.dma_start(out=identb[:], in_=idb_in[:, :])
            scidx = wp.tile([P, NT * NW], I16, name="scidx")
            nc.sync.dma_start(out=scidx[:], in_=scidx_in[:, :])
            giidx = wp.tile([P, NT * NW], I16, name="giidx")
            nc.sync.dma_start(out=giidx[:], in_=giidx_in[:, :])

            # ---- zero the exchange buffers (scatter is +=)
            zt = wp.tile([P, 1024], BF16, name="zt")
            nc.vector.memset(zt[:], 0)
            ZROWS = 1024 * P // DH                       # rows per full zt DMA
            for i in range(DEPTH_ITERS):
                r = 0
                while r < NR:
                    n = min(ZROWS, NR - r)
                    nc.sync.dma_start(out=ccin[i][r:r + n, :],
                                      in_=zt[:, :n * DH // P])
                    r += n

            # ---- persistent SBUF state
            hL = [st.tile([P, KP], BF16, tag=f"hL{f}", name=f"hL{f}") for f in range(NFH)]
            asum = [st.tile([P, APAD], BF16, tag=f"as{f}", name=f"as{f}") for f in range(NFH)]
            asum2 = [st.tile([P, APAD], BF16, tag=f"as2{f}", name=f"as2{f}") for f in range(NFH)]

            def red_segments(c0):
                c1 = c0 + CHUNK
                segs = []
                for r in range(R_ROUNDS):
                    a = max(c0, int(starts[r]))
                    b = min(c1, int(starts[r + 1]))
                    if a >= b:
                        continue
                    segs.append((r, a, b, a - int(starts[r])))
                return segs

            def fused_accum(dst, c0):
                # accumulate freshly written hL[:, c0:c0+CHUNK] into dst by round
                for r, a, b, d0 in red_segments(c0):
                    for f in range(NFH):
                        if r == 0:
                            nc.vector.tensor_copy(out=dst[f][:, d0:d0 + (b - a)],
                                                  in_=hL[f][:, a:b])
                        else:
                            nc.vector.tensor_add(out=dst[f][:, d0:d0 + (b - a)],
                                                 in0=dst[f][:, d0:d0 + (b - a)],
                                                 in1=hL[f][:, a:b])

            def zero_tail(dst):
                n1 = int(n_r[0]) if R_ROUNDS else 0
                for f in range(NFH):
                    nc.vector.memset(dst[f][:, n1:APAD], 0)

            # segments of [c0, c0+CHUNK) by round; last round extends over the
            # pad tail (tail values go to the trash slot anyway)
            def segments(c0):
                c1 = c0 + CHUNK
                segs = []
                for r in range(R_ROUNDS):
                    a = max(c0, int(starts[r]))
                    b = c1 if r == R_ROUNDS - 1 else min(c1, int(starts[r + 1]))
                    if a >= b:
                        continue
                    segs.append((a, b, a - int(starts[r])))
                return segs

            # ---- init: H0 = relu(WiT.T @ X) into resident hL, asum fused
            def init_l():
                for t2 in range((NT + 1) // 2):
                    c0 = t2 * 2 * CHUNK
                    wdt = min(2 * CHUNK, KP - c0)
                    x0 = hp.tile([P, 2 * CHUNK], BF16, tag="x0", name="x0")
                    x1 = hp.tile([DXT - P, 2 * CHUNK], BF16, tag="x1", name="x1")
                    nc.sync.dma_start(out=x0[:, :wdt], in_=x_in[:P, c0:c0 + wdt])
                    nc.sync.dma_start(out=x1[:, :wdt], in_=x_in[P:DXT, c0:c0 + wdt])
                    for half_t in range(wdt // CHUNK):
                        t = t2 * 2 + half_t
                        xo = half_t * CHUNK
                        acc = ps.tile([P, NFH * CHUNK], F32, space="PSUM", tag="acc", name="acc")
                        for f in range(NFH):
                            o = f * CHUNK
                            nc.tensor.matmul(out=acc[:, o:o + CHUNK], lhsT=wiT0[:, f * P:(f + 1) * P],
                                             rhs=x0[:, xo:xo + CHUNK], start=True, stop=False)
                            nc.tensor.matmul(out=acc[:, o:o + CHUNK],
                                             lhsT=wiT1[:, f * P:(f + 1) * P], rhs=x1[:, xo:xo + CHUNK],
                                             start=False, stop=True)
                        for f in range(NFH):
                            nc.scalar.activation(out=hL[f][:, t * CHUNK:(t + 1) * CHUNK],
                                                 in_=acc[:, f * CHUNK:(f + 1) * CHUNK],
                                                 func=mybir.ActivationFunctionType.Relu)
                        fused_accum(asum, t * CHUNK)

            # ---- A4 = C.asum computed in place over asrc (bf16)
            def a4_compute(asrc):
                for t in range(NA_CH):
                    c0 = t * CHUNK
                    c1 = min(c0 + CHUNK, APAD)
                    n = c1 - c0
                    acc = ps.tile([P, NFH * CHUNK], F32, space="PSUM", tag="acc", name="acc")
                    for f in range(NFH):
                        o = f * CHUNK
                        for k in range(NFH):
                            nc.tensor.matmul(out=acc[:, o:o + n],
                                             lhsT=cTb[k][:, f * P:(f + 1) * P],
                                             rhs=asrc[k][:, c0:c1],
                                             start=(k == 0), stop=(k == NFH - 1))
                    for f in range(NFH):
                        nc.vector.tensor_copy(out=asrc[f][:, c0:c1],
                                              in_=acc[:, f * CHUNK:f * CHUNK + n])

            # ---- Vout staging: voutb = WoV.T @ Vfm over chunk range [t0, t1)
            def vout_stage(t0, t1):
                for t in range(t0, t1):
                    c0v = t * CHUNK
                    c1v = min(c0v + CHUNK, APAD)
                    n = c1v - c0v
                    vt0 = hp.tile([P, 2 * CHUNK], BF16, tag="x0", name="xv0")
                    vt1 = hp.tile([DXT - P, 2 * CHUNK], BF16, tag="x1", name="xv1")
                    nc.sync.dma_start(out=vt0[:, :n], in_=vfm_in[:P, c0v:c1v])
                    nc.sync.dma_start(out=vt1[:DV - P, :n], in_=vfm_in[P:DV, c0v:c1v])
                    accv = ps.tile([P, NFH * CHUNK], F32, space="PSUM", tag="acc", name="acc")
                    for f in range(NFH):
                        o = f * CHUNK
                        nc.tensor.matmul(out=accv[:, o:o + n], lhsT=woTv0[:, f * P:(f + 1) * P],
                                         rhs=vt0[:, :n], start=True, stop=False)
                        nc.tensor.matmul(out=accv[:, o:o + n], lhsT=woTv1[:, f * P:(f + 1) * P],
                                         rhs=vt1[:DV - P, :n], start=False, stop=True)
                    for f in range(NFH):
                        vo = rp.tile([P, CHUNK], BF16, tag=f"rr{f}", name=f"vo{f}")
                        nc.scalar.copy(out=vo[:, :n], in_=accv[:, f * CHUNK:f * CHUNK + n])
                        nc.sync.dma_start(out=voutb[f * P:(f + 1) * P, c0v:c1v], in_=vo[:, :n])

            # ================ program ================
            zero_tail(asum)
            init_l()

            for it in range(DEPTH_ITERS):
                asrc = asum if it == 0 else asum2
                adst = asum2 if it == 0 else asum
                a4_compute(asrc)
                zero_tail(adst)

                # ---- T-phase: T = A4[dst] - C.h per column, shipped as rows
                for t in range(NT):
                    c0 = t * CHUNK
                    accT = ps.tile([P, NFH * CHUNK], F32, space="PSUM", tag="acc", name="acc")
                    for f in range(NFH):
                        o = f * CHUNK
                        for k in range(NFH):
                            nc.tensor.matmul(out=accT[:, o:o + CHUNK],
                                             lhsT=cnT[k][:, f * P:(f + 1) * P],
                                             rhs=hL[k][:, c0:c0 + CHUNK],
                                             start=(k == 0), stop=(k == NFH - 1))
                    for f in range(NFH):
                        o = f * CHUNK
                        for a, b, d0 in segments(c0):
                            nc.vector.tensor_add(out=accT[:, o + a - c0:o + b - c0],
                                                 in0=accT[:, o + a - c0:o + b - c0],
                                                 in1=asrc[f][:, d0:d0 + (b - a)])
                    tT = gp.tile([P, NFH, CHUNK], BF16, tag="tT", name="tT")
                    for f in range(NFH):
                        nc.vector.tensor_copy(out=tT[:, f, :],
                                              in_=accT[:, f * CHUNK:(f + 1) * CHUNK])
                    rT = rp.tile([P, CHUNK // P, DH], BF16, tag="rT", name="rT")
                    for b in range(CHUNK // P):
                        tp = ps.tile([P, DH], F32, space="PSUM", tag="tp", name="tp", bufs=2)
                        for f in range(NFH):
                            nc.tensor.matmul(out=tp[:, f * P:(f + 1) * P],
                                             lhsT=tT[:, f, b * P:(b + 1) * P],
                                             rhs=identb[:], start=True, stop=True)
                        nc.vector.tensor_copy(out=rT[:, b, :], in_=tp[:])
                    if t < NTA:
                        out_ap = ccin[it][0:NCORES * BA, :]
                    else:
                        out_ap = ccin[it][NCORES * BA:NR, :]
                    nc.gpsimd.dma_scatter_add(
                        out_ap=out_ap, in_ap=rT[:, :, :],
                        idxs_ap=scidx[:, t * NW:(t + 1) * NW],
                        num_idxs=CHUNK, num_idxs_reg=CHUNK, elem_size=DH,
                    )
                    if t == NTA - 1:
                        nc.gpsimd.collective_compute(
                            "AllToAll", mybir.AluOpType.bypass,
                            replica_groups=[list(range(NCORES))],
                            ins=[ccin[it][0:NCORES * BA, :]],
                            outs=[ccout[it][0:NCORES * BA, :]],
                        )
                nc.gpsimd.collective_compute(
                    "AllToAll", mybir.AluOpType.bypass,
                    replica_groups=[list(range(NCORES))],
                    ins=[ccin[it][NCORES * BA:NR, :]],
                    outs=[ccout[it][NCORES * BA:NR, :]],
                )

                # fill the collective window with independent Vout staging
                if it == 0:
                    vout_stage(0, NA_CH // 2)
                else:
                    vout_stage(NA_CH // 2, NA_CH)

                # ---- U-phase: hL += relu(Wf.hL + T)
                for t in range(NT):
                    c0 = t * CHUNK
                    g = gp.tile([P, NFH, CHUNK], BF16, tag="gg", name="g_t")
                    nc.gpsimd.dma_gather(
                        out_ap=g[:, :, :], in_ap=ccout[it][:, :],
                        idxs_ap=giidx[:, t * NW:(t + 1) * NW],
                        num_idxs=CHUNK, num_idxs_reg=CHUNK, elem_size=DH,
                        transpose=True,
                    )
                    accU = ps.tile([P, NFH * CHUNK], F32, space="PSUM", tag="acc", name="acc")
                    for f in range(NFH):
                        o = f * CHUNK
                        for k in range(NFH):
                            nc.tensor.matmul(out=accU[:, o:o + CHUNK],
                                             lhsT=wfT[k][:, f * P:(f + 1) * P],
                                             rhs=hL[k][:, c0:c0 + CHUNK],
                                             start=(k == 0), stop=(k == NFH - 1))
                    for f in range(NFH):
                        o = f * CHUNK
                        nc.vector.tensor_add(out=accU[:, o:o + CHUNK],
                                             in0=accU[:, o:o + CHUNK], in1=g[:, f, :])
                    for f in range(NFH):
                        rl = rp.tile([P, CHUNK], BF16, tag=f"r{f}", name=f"r{f}")
                        nc.scalar.activation(out=rl[:], in_=accU[:, f * CHUNK:(f + 1) * CHUNK],
                                             func=mybir.ActivationFunctionType.Relu)
                        nc.vector.tensor_add(out=hL[f][:, c0:c0 + CHUNK],
                                             in0=hL[f][:, c0:c0 + CHUNK], in1=rl[:])
                    fused_accum(adst, c0)

            # ---- final: out = relu(Vout + WoS.T @ a_sum)
            for t in range(NA_CH):
                c0 = t * CHUNK
                c1 = min(c0 + CHUNK, APAD)
                n = c1 - c0
                vo = [rp.tile([P, CHUNK], BF16, tag=f"rr{f}", name=f"hr{f}") for f in range(NFH)]
                for f in range(NFH):
                    nc.sync.dma_start(out=vo[f][:, :n], in_=voutb[f * P:(f + 1) * P, c0:c1])
                acc = ps.tile([P, NFH * CHUNK], F32, space="PSUM", tag="acc", name="acc")
                for f in range(NFH):
                    o = f * CHUNK
                    for k in range(NFH):
                        nc.tensor.matmul(out=acc[:, o:o + n], lhsT=woTs[k][:, f * P:(f + 1) * P],
                                         rhs=asum[k][:, c0:c1], start=(k == 0), stop=(k == NFH - 1))
                for f in range(NFH):
                    o = f * CHUNK
                    nc.vector.tensor_add(out=acc[:, o:o + n], in0=acc[:, o:o + n], in1=vo[f][:, :n])
                for f in range(NFH):
                    ot = rp.tile([P, CHUNK], BF16, tag=f"r{f}", name=f"o{f}")
                    nc.scalar.activation(out=ot[:, :n], in_=acc[:, f * CHUNK:f * CHUNK + n],
                                         func=mybir.ActivationFunctionType.Relu)
                    nc.sync.dma_start(out=out_ext[f * P:(f + 1) * P, c0:c1], in_=ot[:, :n])

    nc.compile()
    return nc


# ---------------------------------------------------------------- entry point
def _run(inputs, trace=False):
    from concourse.bass_utils import run_bass_kernel_spmd

    V = np.asarray(inputs["V"], np.float32)
    E_feat = np.asarray(inputs["E_feat"], np.float32)
    meta = _preprocess(V, E_feat, np.asarray(inputs["edge_index"]),
                       np.asarray(inputs["rev_edge_index"]))
    wts = _weights(np.asarray(inputs["Wi_bond"], np.float32),
                   np.asarray(inputs["Wh_bond"], np.float32),
                   np.asarray(inputs["Wf_bond"], np.float32),
                   np.asarray(inputs["Wo_atom"], np.float32), meta["DV"])

    key = (meta["KP"], meta["APAD"], meta["BA"], meta["BB"],
           tuple(meta["starts"].tolist()))
    if key not in _cache:
        _cache[key] = _build(meta)
    nc = _cache[key]

    in_maps = []
    for c in range(NCORES):
        cd = meta["cores"][c]
        in_maps.append({
            "x": cd["X"], "vfm": cd["Vfm"], "scidx": cd["scidx"], "giidx": cd["giidx"],
            "wiT": wts["WiT"], "wfT": wts["WfT"], "cT": wts["CT"], "cnT": wts["CnT"],
            "woTv": wts["WoTv"], "woTs": wts["WoTs"], "identb": wts["identb"],
        })
    res = run_bass_kernel_spmd(nc, in_maps, core_ids=list(range(NCORES)), trace=trace)

    N, ASH = meta["N"], meta["ASH"]
    out = np.empty((N, DH), np.float32)
    for c in range(NCORES):
        o = np.asarray(res.results[c]["out"]).astype(np.float32)   # [DH, APAD]
        order = meta["cores"][c]["order"]
        out[c * ASH + order] = o[:, :ASH].T
    return out, res.exec_time_ns


def kernel(**inputs) -> np.ndarray:
    out, _ = _run(inputs, trace=False)
    return out


# revision 19
# speedup vs baseline: 1.0196x; 1.0196x over previous
"""Trainium2 Bass kernel for CommunicativeMessagePassing (D-MPNN bond-message GNN).

Self-contained: takes full inputs, shards across 8 NeuronCores, returns full output.

Math (dead code removed -- the reference's H_a / a_max / gate chain never reaches
the output):
    H_b = relu(concat(V[v], E_feat) @ Wi_bond.T)
    2x:  a_sum = segment_sum(H_b, w); H_b += relu((H_b + (a_sum[v] - H_b[rev]) @ Wh.T) @ Wf.T)
    a_sum = segment_sum(H_b, w); out = relu(concat(V, a_sum) @ Wo_atom.T)

v3 (T-exchange): every edge update is computed exactly ONCE, on the core owning
its destination atom. Per-edge update relu(Wf.h_e + A4[v_e] - C.h_rev(e)) with
C = Wf@Wh needs A4[v_e] - C.h_rev(e), both local to the core owning rev(e)
(dst of rev(e) is v_e). That core computes T_rev = A4[dst] - C.h for each of its
edges and ships the 512B row to the home core of the paired edge through
dma_scatter_add -> AllToAll -> dma_gather(transpose=True). The transposing
gather lands T directly feature-major, so the receive side is a plain PSUM add.
No mirrored R-state, no R-half init, no A4 AllGather, no transpose matmuls on
the receive path.

Sharding: atoms split into 8 contiguous ranges; a core owns the edges whose dst
atom is local, columns in round-major degree-sorted order for dense segment-sum.
The AllToAll is split in two (by sender column half) so the first half flies
while the second half's T is still being computed.
"""
import sys
sys.path.insert(0, "/opt/trn_rl_repo")
import numpy as np
import ml_dtypes
BF = ml_dtypes.bfloat16

NCORES = 8
P = 128
CHUNK = 512
DH = 256

_cache = {}


def _wrap_idx(idx):
    """Pack an index list into the [128, n/16] wrapped+replicated int16 layout."""
    a = np.asarray(idx, np.int16).reshape(-1, 16).T          # [16, n/16]
    return np.tile(a, (8, 1)).copy()


# ---------------------------------------------------------------- host preprocessing
def _preprocess(V, E_feat, edge_index, rev_edge_index):
    N, DV = V.shape
    E, DE = E_feat.shape
    v = np.asarray(edge_index[0], np.int64)
    w = np.asarray(edge_index[1], np.int64)
    rev = np.asarray(rev_edge_index, np.int64)
    ASH = N // NCORES
    APAD = ((ASH + P - 1) // P) * P

    shard = w // ASH

    per = []
    for c in range(NCORES):
        eids = np.nonzero(shard == c)[0]
        wl = w[eids] - c * ASH
        deg = np.bincount(wl, minlength=ASH)
        order = np.argsort(-deg, kind="stable")      # rank -> atom (local)
        rank_of = np.empty(ASH, np.int64)
        rank_of[order] = np.arange(ASH)
        ar = rank_of[wl]
        o2 = np.lexsort((eids, ar))
        eids_s, ar_s = eids[o2], ar[o2]
        if len(ar_s):
            runs = np.r_[0, np.nonzero(np.diff(ar_s))[0] + 1]
            lens = np.diff(np.r_[runs, len(ar_s)])
            pos = np.arange(len(ar_s)) - np.repeat(runs, lens)
        else:
            pos = np.zeros(0, np.int64)
        per.append(dict(eids=eids_s, ar=ar_s, pos=pos, deg=deg, order=order))

    maxdeg = max(int(p["deg"].max()) for p in per) if E else 0
    n_r = np.zeros(maxdeg, np.int64)
    for p in per:
        cnt = np.bincount(p["deg"], minlength=maxdeg + 1)
        gt = ASH - np.cumsum(cnt)[:maxdeg]
        n_r = np.maximum(n_r, gt)
    starts = np.r_[0, np.cumsum(n_r)]
    K = int(starts[-1])
    KP = ((K + CHUNK - 1) // CHUNK) * CHUNK
    NT = KP // CHUNK
    NTA = (NT // 2) & ~1                 # even chunk count in exchange region A
    HALF = NTA * CHUNK

    # per-core column tables
    for c in range(NCORES):
        p = per[c]
        cols = starts[p["pos"]] + p["ar"]
        L_eid = np.full(KP, -1, np.int64)
        L_eid[cols] = p["eids"]
        p["L_eid"] = L_eid
        p["dd"] = np.where(L_eid >= 0, v[np.maximum(L_eid, 0)] // ASH, -1)

    # exchange block sizes: max rows for any (sender, dest) pair per column half
    maxA = maxB = 0
    for c in range(NCORES):
        dd, Le = per[c]["dd"], per[c]["L_eid"]
        for d in range(NCORES):
            mA = (dd == d) & (np.arange(KP) < HALF)
            mB = (dd == d) & (np.arange(KP) >= HALF)
            maxA = max(maxA, int(mA.sum()))
            maxB = max(maxB, int(mB.sum()))
    BA, BB = maxA + 2, maxB + 2          # +trash slot, +guaranteed-zero slot
    NR = NCORES * (BA + BB)
    assert NR < 32768, NR
    ZERO_ROW = BA - 2                    # sender-0 region-A zero slot

    # sender slot assignment + global row of each edge's T value
    rowtab = np.full(E, -1, np.int64)
    for c in range(NCORES):
        dd, Le = per[c]["dd"], per[c]["L_eid"]
        SC = np.empty(KP, np.int64)
        colr = np.arange(KP)
        for base, Breg, m_reg, gbase in ((0, BA, colr < HALF, 0),
                                         (0, BB, colr >= HALF, NCORES * BA)):
            SC[m_reg] = Breg - 1         # pads -> region trash slot (block 0)
            for d in range(NCORES):
                m = (dd == d) & m_reg
                idxs = np.nonzero(m)[0]
                SC[idxs] = d * Breg + np.arange(len(idxs))
                # receiver-side row: AllToAll puts sender c's block at c*Breg
                rowtab[Le[idxs]] = gbase + c * Breg + np.arange(len(idxs))
        per[c]["SC"] = SC

    cores = []
    for c in range(NCORES):
        p = per[c]
        L_eid = p["L_eid"]
        mask = L_eid >= 0
        GI = np.full(KP, ZERO_ROW, np.int64)
        GI[mask] = rowtab[rev[L_eid[mask]]]
        assert GI.min() >= 0 and GI.max() < NR

        # X staging, feature-major [DV+DE, KP]
        X = np.zeros((DV + DE, KP), BF)
        le = L_eid[mask]
        X[:DV][:, mask] = V[v[le]].T.astype(BF)
        X[DV:][:, mask] = E_feat[le].T.astype(BF)

        Vfm = np.zeros((DV, APAD), BF)
        Vfm[:, :ASH] = V[c * ASH + p["order"]].T.astype(BF)
        cores.append(dict(scidx=_wrap_idx(p["SC"]), giidx=_wrap_idx(GI),
                          X=X, Vfm=Vfm, order=p["order"]))

    return dict(N=N, E=E, DV=DV, DE=DE, ASH=ASH, APAD=APAD, KP=KP,
                BA=BA, BB=BB, NR=NR, NTA=NTA,
                starts=starts.astype(np.int64), n_r=n_r, cores=cores)


def _weights(Wi_bond, Wh_bond, Wf_bond, Wo_atom, DV):
    C = (Wf_bond @ Wh_bond).astype(np.float32)
    return dict(
        WiT=np.ascontiguousarray(Wi_bond.T.astype(BF)),       # [DV+DE, DH]
        WfT=np.ascontiguousarray(Wf_bond.T.astype(BF)),       # [DH, DH]
        CT=np.ascontiguousarray(C.T.astype(BF)),
        CnT=np.ascontiguousarray((-C.T).astype(BF)),
        WoTv=np.ascontiguousarray(Wo_atom.T[:DV].astype(BF)),  # [DV, DH]
        WoTs=np.ascontiguousarray(Wo_atom.T[DV:].astype(BF)),  # [DH, DH]
        identb=np.eye(P, dtype=BF),
    )


# ---------------------------------------------------------------- bass program
def _build(meta, DEPTH_ITERS=2):
    import concourse.bass as bass
    import concourse.bacc as bacc
    import concourse.tile as tile
    from concourse import mybir

    F32 = mybir.dt.float32
    BF16 = mybir.dt.bfloat16
    I16 = mybir.dt.int16
    KP, APAD, DV, DE = meta["KP"], meta["APAD"], meta["DV"], meta["DE"]
    BA, BB, NR = meta["BA"], meta["BB"], meta["NR"]
    starts, n_r = meta["starts"], meta["n_r"]
    R_ROUNDS = len(n_r)
    DXT = DV + DE
    NFH = DH // P                      # feature halves (2)
    NA_CH = (APAD + CHUNK - 1) // CHUNK
    NT = KP // CHUNK
    NTA = meta["NTA"]                  # chunks in exchange region A (even)
    NW = CHUNK // 16                   # idx columns per chunk (32)
    SCB = 1                            # chunks per scatter call
    GB = 1                             # chunks per gather call

    nc = bacc.Bacc("TRN2", target_bir_lowering=False, debug=False, num_devices=NCORES)

    x_in = nc.dram_tensor("x", [DXT, KP], BF16, kind="ExternalInput")
    vfm_in = nc.dram_tensor("vfm", [DV, APAD], BF16, kind="ExternalInput")
    scidx_in = nc.dram_tensor("scidx", [P, NT * NW], I16, kind="ExternalInput")
    giidx_in = nc.dram_tensor("giidx", [P, NT * NW], I16, kind="ExternalInput")
    wiT_in = nc.dram_tensor("wiT", [DXT, DH], BF16, kind="ExternalInput")
    wfT_in = nc.dram_tensor("wfT", [DH, DH], BF16, kind="ExternalInput")
    ct_in = nc.dram_tensor("cT", [DH, DH], BF16, kind="ExternalInput")
    cnT_in = nc.dram_tensor("cnT", [DH, DH], BF16, kind="ExternalInput")
    woTv_in = nc.dram_tensor("woTv", [DV, DH], BF16, kind="ExternalInput")
    woTs_in = nc.dram_tensor("woTs", [DH, DH], BF16, kind="ExternalInput")
    idb_in = nc.dram_tensor("identb", [P, P], BF16, kind="ExternalInput")
    out_ext = nc.dram_tensor("out", [DH, APAD], BF16, kind="ExternalOutput")

    voutb = nc.dram_tensor("voutb", [DH, APAD], BF16)
    ccin = [nc.dram_tensor(f"ccin{i}", [NR, DH], BF16) for i in range(DEPTH_ITERS)]
    ccout = [nc.dram_tensor(f"ccout{i}", [NR, DH], BF16) for i in range(DEPTH_ITERS)]

    with tile.TileContext(nc) as tc:
        with (
            tc.tile_pool(name="wpool", bufs=1) as wp,
            tc.tile_pool(name="state", bufs=1) as st,
            tc.tile_pool(name="hin", bufs=2) as hp,
            tc.tile_pool(name="rout", bufs=2) as rp,
            tc.tile_pool(name="gpool", bufs=2) as gp,
            tc.tile_pool(name="psum", bufs=3, space="PSUM") as ps,
        ):
            # ---- resident weights
            wiT0 = wp.tile([P, DH], BF16, name="wiT0")
            nc.sync.dma_start(out=wiT0[:], in_=wiT_in[:P, :])
            wiT1 = wp.tile([DXT - P, DH], BF16, name="wiT1")
            nc.sync.dma_start(out=wiT1[:], in_=wiT_in[P:DXT, :])
            wfT = [wp.tile([P, DH], BF16, tag=f"wf{k}", name=f"wf{k}") for k in range(NFH)]
            cTb = [wp.tile([P, DH], BF16, tag=f"ct{k}", name=f"ct{k}") for k in range(NFH)]
            cnT = [wp.tile([P, DH], BF16, tag=f"cn{k}", name=f"cn{k}") for k in range(NFH)]
            for k in range(NFH):
                nc.sync.dma_start(out=wfT[k][:], in_=wfT_in[k * P:(k + 1) * P, :])
                nc.sync.dma_start(out=cTb[k][:], in_=ct_in[k * P:(k + 1) * P, :])
                nc.sync.dma_start(out=cnT[k][:], in_=cnT_in[k * P:(k + 1) * P, :])
            woTv0 = wp.tile([P, DH], BF16, name="woTv0")
            nc.sync.dma_start(out=woTv0[:], in_=woTv_in[:P, :])
            woTv1 = wp.tile([DV - P, DH], BF16, name="woTv1")
            nc.sync.dma_start(out=woTv1[:], in_=woTv_in[P:DV, :])
            woTs = [wp.tile([P, DH], BF16, tag=f"wo{k}", name=f"wo{k}") for k in range(NFH)]
            for k in range(NFH):
                nc.sync.dma_start(out=woTs[k][:], in_=woTs_in[k * P:(k + 1) * P, :])
            identb = wp.tile([P, P], BF16, name="identb")
            nc.sync.dma_start(out=identb[:], in_=idb_in[:, :])
            scidx = wp.tile([P, NT * NW], I16, name="scidx")
            nc.sync.dma_start(out=scidx[:], in_=scidx_in[:, :])
            giidx = wp.tile([P, NT * NW], I16, name="giidx")
            nc.sync.dma_start(out=giidx[:], in_=giidx_in[:, :])

            # ---- persistent SBUF state
            hL = [st.tile([P, KP], BF16, tag=f"hL{f}", name=f"hL{f}") for f in range(NFH)]
            asum = [st.tile([P, APAD], BF16, tag=f"as{f}", name=f"as{f}") for f in range(NFH)]
            asum2 = [st.tile([P, APAD], BF16, tag=f"as2{f}", name=f"as2{f}") for f in range(NFH)]

            def red_segments(c0):
                c1 = c0 + CHUNK
                segs = []
                for r in range(R_ROUNDS):
                    a = max(c0, int(starts[r]))
                    b = min(c1, int(starts[r + 1]))
                    if a >= b:
                        continue
                    segs.append((r, a, b, a - int(starts[r])))
                return segs

            def fused_accum(dst, c0):
                # accumulate freshly written hL[:, c0:c0+CHUNK] into dst by round
                for r, a, b, d0 in red_segments(c0):
                    for f in range(NFH):
                        if r == 0:
                            nc.vector.tensor_copy(out=dst[f][:, d0:d0 + (b - a)],
                                                  in_=hL[f][:, a:b])
                        else:
                            nc.vector.tensor_add(out=dst[f][:, d0:d0 + (b - a)],
                                                 in0=dst[f][:, d0:d0 + (b - a)],
                                                 in1=hL[f][:, a:b])

            def zero_tail(dst):
                n1 = int(n_r[0]) if R_ROUNDS else 0
                for f in range(NFH):
                    nc.vector.memset(dst[f][:, n1:APAD], 0)

            # segments of [c0, c0+CHUNK) by round; last round extends over the
            # pad tail (tail values go to the trash slot anyway)
            def segments(c0):
                c1 = c0 + CHUNK
                segs = []
                for r in range(R_ROUNDS):
                    a = max(c0, int(starts[r]))
                    b = c1 if r == R_ROUNDS - 1 else min(c1, int(starts[r + 1]))
                    if a >= b:
                        continue
                    segs.append((a, b, a - int(starts[r])))
                return segs

            # ---- init: H0 = relu(WiT.T @ X) into resident hL, asum fused
            def init_l():
                for t2 in range((NT + 1) // 2):
                    c0 = t2 * 2 * CHUNK
                    wdt = min(2 * CHUNK, KP - c0)
                    x0 = hp.tile([P, 2 * CHUNK], BF16, tag="x0", name="x0")
                    x1 = hp.tile([DXT - P, 2 * CHUNK], BF16, tag="x1", name="x1")
                    nc.sync.dma_start(out=x0[:, :wdt], in_=x_in[:P, c0:c0 + wdt])
                    nc.sync.dma_start(out=x1[:, :wdt], in_=x_in[P:DXT, c0:c0 + wdt])
                    for half_t in range(wdt // CHUNK):
                        t = t2 * 2 + half_t
                        xo = half_t * CHUNK
                        acc = ps.tile([P, NFH * CHUNK], F32, space="PSUM", tag="acc", name="acc")
                        for f in range(NFH):
                            o = f * CHUNK
                            nc.tensor.matmul(out=acc[:, o:o + CHUNK], lhsT=wiT0[:, f * P:(f + 1) * P],
                                             rhs=x0[:, xo:xo + CHUNK], start=True, stop=False)
                            nc.tensor.matmul(out=acc[:, o:o + CHUNK],
                                             lhsT=wiT1[:, f * P:(f + 1) * P], rhs=x1[:, xo:xo + CHUNK],
                                             start=False, stop=True)
                        for f in range(NFH):
                            nc.scalar.activation(out=hL[f][:, t * CHUNK:(t + 1) * CHUNK],
                                                 in_=acc[:, f * CHUNK:(f + 1) * CHUNK],
                                                 func=mybir.ActivationFunctionType.Relu)
                        fused_accum(asum, t * CHUNK)

            # ---- A4 = C.asum computed in place over asrc (bf16)
            def a4_compute(asrc):
                for t in range(NA_CH):
                    c0 = t * CHUNK
                    c1 = min(c0 + CHUNK, APAD)
                    n = c1 - c0
                    acc = ps.tile([P, NFH * CHUNK], F32, space="PSUM", tag="acc", name="acc")
                    for f in range(NFH):
                        o = f * CHUNK
                        for k in range(NFH):
                            nc.tensor.matmul(out=acc[:, o:o + n],
                                             lhsT=cTb[k][:, f * P:(f + 1) * P],
                                             rhs=asrc[k][:, c0:c1],
                                             start=(k == 0), stop=(k == NFH - 1))
                    for f in range(NFH):
                        nc.vector.tensor_copy(out=asrc[f][:, c0:c1],
                                              in_=acc[:, f * CHUNK:f * CHUNK + n])

            # ---- Vout staging: voutb = WoV.T @ Vfm over chunk range [t0, t1)
            def vout_stage(t0, t1):
                for t in range(t0, t1):
                    c0v = t * CHUNK
                    c1v = min(c0v + CHUNK, APAD)
                    n = c1v - c0v
                    vt0 = hp.tile([P, 2 * CHUNK], BF16, tag="x0", name="xv0")
                    vt1 = hp.tile([DXT - P, 2 * CHUNK], BF16, tag="x1", name="xv1")
                    nc.sync.dma_start(out=vt0[:, :n], in_=vfm_in[:P, c0v:c1v])
                    nc.sync.dma_start(out=vt1[:DV - P, :n], in_=vfm_in[P:DV, c0v:c1v])
                    accv = ps.tile([P, NFH * CHUNK], F32, space="PSUM", tag="acc", name="acc")
                    for f in range(NFH):
                        o = f * CHUNK
                        nc.tensor.matmul(out=accv[:, o:o + n], lhsT=woTv0[:, f * P:(f + 1) * P],
                                         rhs=vt0[:, :n], start=True, stop=False)
                        nc.tensor.matmul(out=accv[:, o:o + n], lhsT=woTv1[:, f * P:(f + 1) * P],
                                         rhs=vt1[:DV - P, :n], start=False, stop=True)
                    for f in range(NFH):
                        vo = rp.tile([P, CHUNK], BF16, tag=f"rr{f}", name=f"vo{f}")
                        nc.scalar.copy(out=vo[:, :n], in_=accv[:, f * CHUNK:f * CHUNK + n])
                        nc.sync.dma_start(out=voutb[f * P:(f + 1) * P, c0v:c1v], in_=vo[:, :n])

            # ================ program ================
            zero_tail(asum)
            init_l()

            # zero the exchange buffers (scatter is +=); emitted after init so
            # the writes ride on otherwise-idle queues during init compute
            zt = wp.tile([P, 1024], BF16, name="zt")
            nc.vector.memset(zt[:], 0)
            ZROWS = 1024 * P // DH                       # rows per full zt DMA
            for i in range(DEPTH_ITERS):
                r = 0
                while r < NR:
                    n = min(ZROWS, NR - r)
                    nc.sync.dma_start(out=ccin[i][r:r + n, :],
                                      in_=zt[:, :n * DH // P])
                    r += n

            for it in range(DEPTH_ITERS):
                asrc = asum if it == 0 else asum2
                adst = asum2 if it == 0 else asum
                a4_compute(asrc)
                zero_tail(adst)

                # ---- T-phase: T = A4[dst] - C.h per column, shipped as rows
                for t in range(NT):
                    c0 = t * CHUNK
                    q = t % SCB                          # slot within scatter batch
                    accT = ps.tile([P, NFH * CHUNK], F32, space="PSUM", tag="acc", name="acc")
                    for f in range(NFH):
                        o = f * CHUNK
                        for k in range(NFH):
                            nc.tensor.matmul(out=accT[:, o:o + CHUNK],
                                             lhsT=cnT[k][:, f * P:(f + 1) * P],
                                             rhs=hL[k][:, c0:c0 + CHUNK],
                                             start=(k == 0), stop=(k == NFH - 1))
                    for f in range(NFH):
                        o = f * CHUNK
                        for a, b, d0 in segments(c0):
                            nc.vector.tensor_add(out=accT[:, o + a - c0:o + b - c0],
                                                 in0=accT[:, o + a - c0:o + b - c0],
                                                 in1=asrc[f][:, d0:d0 + (b - a)])
                    tT = gp.tile([P, NFH, CHUNK], BF16, tag="tT", name="tT")
                    for f in range(NFH):
                        nc.vector.tensor_copy(out=tT[:, f, :],
                                              in_=accT[:, f * CHUNK:(f + 1) * CHUNK])
                    if q == 0:
                        rT = rp.tile([P, SCB * CHUNK // P, DH], BF16, tag="rT", name="rT")
                    for b in range(CHUNK // P):
                        tp = ps.tile([P, DH], F32, space="PSUM", tag="tp", name="tp", bufs=2)
                        for f in range(NFH):
                            nc.tensor.matmul(out=tp[:, f * P:(f + 1) * P],
                                             lhsT=tT[:, f, b * P:(b + 1) * P],
                                             rhs=identb[:], start=True, stop=True)
                        nc.scalar.copy(out=rT[:, q * (CHUNK // P) + b, :], in_=tp[:])
                    if q == SCB - 1:
                        t0 = t - (SCB - 1)
                        if t < NTA:
                            out_ap = ccin[it][0:NCORES * BA, :]
                        else:
                            out_ap = ccin[it][NCORES * BA:NR, :]
                        nc.gpsimd.dma_scatter_add(
                            out_ap=out_ap, in_ap=rT[:, :, :],
                            idxs_ap=scidx[:, t0 * NW:(t + 1) * NW],
                            num_idxs=SCB * CHUNK, num_idxs_reg=SCB * CHUNK, elem_size=DH,
                        )
                    if t == NTA - 1:
                        nc.gpsimd.collective_compute(
                            "AllToAll", mybir.AluOpType.bypass,
                            replica_groups=[list(range(NCORES))],
                            ins=[ccin[it][0:NCORES * BA, :]],
                            outs=[ccout[it][0:NCORES * BA, :]],
                        )
                nc.gpsimd.collective_compute(
                    "AllToAll", mybir.AluOpType.bypass,
                    replica_groups=[list(range(NCORES))],
                    ins=[ccin[it][NCORES * BA:NR, :]],
                    outs=[ccout[it][NCORES * BA:NR, :]],
                )

                # fill the collective window with independent Vout staging
                if it == 0:
                    vout_stage(0, NA_CH // 2)
                else:
                    vout_stage(NA_CH // 2, NA_CH)

                # ---- U-phase: hL += relu(Wf.hL + T)
                for t in range(NT):
                    c0 = t * CHUNK
                    gq = t % GB
                    if gq == 0:
                        gn = min(GB, NT - t)
                        g = gp.tile([P, NFH, gn * CHUNK], BF16, tag="gg", name="g_t")
                        nc.gpsimd.dma_gather(
                            out_ap=g[:, :, :], in_ap=ccout[it][:, :],
                            idxs_ap=giidx[:, t * NW:(t + gn) * NW],
                            num_idxs=gn * CHUNK, num_idxs_reg=gn * CHUNK, elem_size=DH,
                            transpose=True,
                        )
                    accU = ps.tile([P, NFH * CHUNK], F32, space="PSUM", tag="acc", name="acc")
                    for f in range(NFH):
                        o = f * CHUNK
                        for k in range(NFH):
                            nc.tensor.matmul(out=accU[:, o:o + CHUNK],
                                             lhsT=wfT[k][:, f * P:(f + 1) * P],
                                             rhs=hL[k][:, c0:c0 + CHUNK],
                                             start=(k == 0), stop=(k == NFH - 1))
                    for f in range(NFH):
                        o = f * CHUNK
                        nc.vector.tensor_add(out=accU[:, o:o + CHUNK],
                                             in0=accU[:, o:o + CHUNK],
                                             in1=g[:, f, gq * CHUNK:(gq + 1) * CHUNK])
                    for f in range(NFH):
                        rl = rp.tile([P, CHUNK], BF16, tag=f"r{f}", name=f"r{f}")
                        nc.scalar.activation(out=rl[:], in_=accU[:, f * CHUNK:(f + 1) * CHUNK],
                                             func=mybir.ActivationFunctionType.Relu)
                        nc.vector.tensor_add(out=hL[f][:, c0:c0 + CHUNK],
                                             in0=hL[f][:, c0:c0 + CHUNK], in1=rl[:])
                    fused_accum(adst, c0)

            # ---- final: out = relu(Vout + WoS.T @ a_sum)
            for t in range(NA_CH):
                c0 = t * CHUNK
                c1 = min(c0 + CHUNK, APAD)
                n = c1 - c0
                vo = [rp.tile([P, CHUNK], BF16, tag=f"rr{f}", name=f"hr{f}") for f in range(NFH)]
                for f in range(NFH):
                    nc.sync.dma_start(out=vo[f][:, :n], in_=voutb[f * P:(f + 1) * P, c0:c1])
                acc = ps.tile([P, NFH * CHUNK], F32, space="PSUM", tag="acc", name="acc")
                for f in range(NFH):
                    o = f * CHUNK
                    for k in range(NFH):
                        nc.tensor.matmul(out=acc[:, o:o + n], lhsT=woTs[k][:, f * P:(f + 1) * P],
                                         rhs=asum[k][:, c0:c1], start=(k == 0), stop=(k == NFH - 1))
                for f in range(NFH):
                    o = f * CHUNK
                    nc.vector.tensor_add(out=acc[:, o:o + n], in0=acc[:, o:o + n], in1=vo[f][:, :n])
                for f in range(NFH):
                    ot = rp.tile([P, CHUNK], BF16, tag=f"r{f}", name=f"o{f}")
                    nc.scalar.activation(out=ot[:, :n], in_=acc[:, f * CHUNK:f * CHUNK + n],
                                         func=mybir.ActivationFunctionType.Relu)
                    nc.sync.dma_start(out=out_ext[f * P:(f + 1) * P, c0:c1], in_=ot[:, :n])

    nc.compile()
    return nc


# ---------------------------------------------------------------- entry point
def _run(inputs, trace=False):
    from concourse.bass_utils import run_bass_kernel_spmd

    V = np.asarray(inputs["V"], np.float32)
    E_feat = np.asarray(inputs["E_feat"], np.float32)
    meta = _preprocess(V, E_feat, np.asarray(inputs["edge_index"]),
                       np.asarray(inputs["rev_edge_index"]))
    wts = _weights(np.asarray(inputs["Wi_bond"], np.float32),
                   np.asarray(inputs["Wh_bond"], np.float32),
                   np.asarray(inputs["Wf_bond"], np.float32),
                   np.asarray(inputs["Wo_atom"], np.float32), meta["DV"])

    key = (meta["KP"], meta["APAD"], meta["BA"], meta["BB"],
           tuple(meta["starts"].tolist()))
    if key not in _cache:
        _cache[key] = _build(meta)
    nc = _cache[key]

    in_maps = []
    for c in range(NCORES):
        cd = meta["cores"][c]
        in_maps.append({
            "x": cd["X"], "vfm": cd["Vfm"], "scidx": cd["scidx"], "giidx": cd["giidx"],
            "wiT": wts["WiT"], "wfT": wts["WfT"], "cT": wts["CT"], "cnT": wts["CnT"],
            "woTv": wts["WoTv"], "woTs": wts["WoTs"], "identb": wts["identb"],
        })
    res = run_bass_kernel_spmd(nc, in_maps, core_ids=list(range(NCORES)), trace=trace)

    N, ASH = meta["N"], meta["ASH"]
    out = np.empty((N, DH), np.float32)
    for c in range(NCORES):
        o = np.asarray(res.results[c]["out"]).astype(np.float32)   # [DH, APAD]
        order = meta["cores"][c]["order"]
        out[c * ASH + order] = o[:, :ASH].T
    return out, res.exec_time_ns


def kernel(**inputs) -> np.ndarray:
    out, _ = _run(inputs, trace=False)
    return out


# revision 20
# speedup vs baseline: 1.0616x; 1.0412x over previous
"""Trainium2 Bass kernel for CommunicativeMessagePassing (D-MPNN bond-message GNN).

Self-contained: takes full inputs, shards across 8 NeuronCores, returns full output.

Math (dead code removed -- the reference's H_a / a_max / gate chain never reaches
the output):
    H_b = relu(concat(V[v], E_feat) @ Wi_bond.T)
    2x:  a_sum = segment_sum(H_b, w); H_b += relu((H_b + (a_sum[v] - H_b[rev]) @ Wh.T) @ Wf.T)
    a_sum = segment_sum(H_b, w); out = relu(concat(V, a_sum) @ Wo_atom.T)

v3 (T-exchange): every edge update is computed exactly ONCE, on the core owning
its destination atom. Per-edge update relu(Wf.h_e + A4[v_e] - C.h_rev(e)) with
C = Wf@Wh needs A4[v_e] - C.h_rev(e), both local to the core owning rev(e)
(dst of rev(e) is v_e). That core computes T_rev = A4[dst] - C.h for each of its
edges and ships the 512B row to the home core of the paired edge through
dma_scatter_add -> AllToAll -> dma_gather(transpose=True). The transposing
gather lands T directly feature-major, so the receive side is a plain PSUM add.
No mirrored R-state, no R-half init, no A4 AllGather, no transpose matmuls on
the receive path.

Sharding: atoms split into 8 contiguous ranges; a core owns the edges whose dst
atom is local, columns in round-major degree-sorted order for dense segment-sum.
The AllToAll is split in two (by sender column half) so the first half flies
while the second half's T is still being computed.
"""
import sys
sys.path.insert(0, "/opt/trn_rl_repo")
import numpy as np
import ml_dtypes
BF = ml_dtypes.bfloat16

NCORES = 8
P = 128
CHUNK = 512
DH = 256

_cache = {}


def _wrap_idx(idx):
    """Pack an index list into the [128, n/16] wrapped+replicated int16 layout."""
    a = np.asarray(idx, np.int16).reshape(-1, 16).T          # [16, n/16]
    return np.tile(a, (8, 1)).copy()


# ---------------------------------------------------------------- host preprocessing
def _preprocess(V, E_feat, edge_index, rev_edge_index):
    N, DV = V.shape
    E, DE = E_feat.shape
    v = np.asarray(edge_index[0], np.int64)
    w = np.asarray(edge_index[1], np.int64)
    rev = np.asarray(rev_edge_index, np.int64)
    ASH = N // NCORES
    APAD = ((ASH + P - 1) // P) * P

    shard = w // ASH

    per = []
    for c in range(NCORES):
        eids = np.nonzero(shard == c)[0]
        wl = w[eids] - c * ASH
        deg = np.bincount(wl, minlength=ASH)
        order = np.argsort(-deg, kind="stable")      # rank -> atom (local)
        rank_of = np.empty(ASH, np.int64)
        rank_of[order] = np.arange(ASH)
        ar = rank_of[wl]
        o2 = np.lexsort((eids, ar))
        eids_s, ar_s = eids[o2], ar[o2]
        if len(ar_s):
            runs = np.r_[0, np.nonzero(np.diff(ar_s))[0] + 1]
            lens = np.diff(np.r_[runs, len(ar_s)])
            pos = np.arange(len(ar_s)) - np.repeat(runs, lens)
        else:
            pos = np.zeros(0, np.int64)
        per.append(dict(eids=eids_s, ar=ar_s, pos=pos, deg=deg, order=order))

    maxdeg = max(int(p["deg"].max()) for p in per) if E else 0
    n_r = np.zeros(maxdeg, np.int64)
    for p in per:
        cnt = np.bincount(p["deg"], minlength=maxdeg + 1)
        gt = ASH - np.cumsum(cnt)[:maxdeg]
        n_r = np.maximum(n_r, gt)
    starts = np.r_[0, np.cumsum(n_r)]
    K = int(starts[-1])
    KP = ((K + CHUNK - 1) // CHUNK) * CHUNK
    NT = KP // CHUNK
    NTA = (NT // 2) & ~1                 # even chunk count in exchange region A
    HALF = NTA * CHUNK

    # per-core column tables
    for c in range(NCORES):
        p = per[c]
        cols = starts[p["pos"]] + p["ar"]
        L_eid = np.full(KP, -1, np.int64)
        L_eid[cols] = p["eids"]
        p["L_eid"] = L_eid
        p["dd"] = np.where(L_eid >= 0, v[np.maximum(L_eid, 0)] // ASH, -1)

    # exchange block sizes: max rows for any (sender, dest) pair per column half
    maxA = maxB = 0
    for c in range(NCORES):
        dd, Le = per[c]["dd"], per[c]["L_eid"]
        for d in range(NCORES):
            mA = (dd == d) & (np.arange(KP) < HALF)
            mB = (dd == d) & (np.arange(KP) >= HALF)
            maxA = max(maxA, int(mA.sum()))
            maxB = max(maxB, int(mB.sum()))
    BA, BB = maxA + 2, maxB + 2          # +trash slot, +guaranteed-zero slot
    NR = NCORES * (BA + BB)
    assert NR < 32768, NR
    ZERO_ROW = BA - 2                    # sender-0 region-A zero slot

    # sender slot assignment + global row of each edge's T value
    rowtab = np.full(E, -1, np.int64)
    for c in range(NCORES):
        dd, Le = per[c]["dd"], per[c]["L_eid"]
        SC = np.empty(KP, np.int64)
        colr = np.arange(KP)
        for base, Breg, m_reg, gbase in ((0, BA, colr < HALF, 0),
                                         (0, BB, colr >= HALF, NCORES * BA)):
            SC[m_reg] = Breg - 1         # pads -> region trash slot (block 0)
            for d in range(NCORES):
                m = (dd == d) & m_reg
                idxs = np.nonzero(m)[0]
                SC[idxs] = d * Breg + np.arange(len(idxs))
                # receiver-side row: AllToAll puts sender c's block at c*Breg
                rowtab[Le[idxs]] = gbase + c * Breg + np.arange(len(idxs))
        per[c]["SC"] = SC

    cores = []
    for c in range(NCORES):
        p = per[c]
        L_eid = p["L_eid"]
        mask = L_eid >= 0
        GI = np.full(KP, ZERO_ROW, np.int64)
        GI[mask] = rowtab[rev[L_eid[mask]]]
        assert GI.min() >= 0 and GI.max() < NR

        # X staging, feature-major [DV+DE, KP]
        X = np.zeros((DV + DE, KP), BF)
        le = L_eid[mask]
        X[:DV][:, mask] = V[v[le]].T.astype(BF)
        X[DV:][:, mask] = E_feat[le].T.astype(BF)

        Vfm = np.zeros((DV, APAD), BF)
        Vfm[:, :ASH] = V[c * ASH + p["order"]].T.astype(BF)
        cores.append(dict(scidx=_wrap_idx(p["SC"]), giidx=_wrap_idx(GI),
                          X=X, Vfm=Vfm, order=p["order"]))

    return dict(N=N, E=E, DV=DV, DE=DE, ASH=ASH, APAD=APAD, KP=KP,
                BA=BA, BB=BB, NR=NR, NTA=NTA,
                starts=starts.astype(np.int64), n_r=n_r, cores=cores)


def _weights(Wi_bond, Wh_bond, Wf_bond, Wo_atom, DV):
    C = (Wf_bond @ Wh_bond).astype(np.float32)
    return dict(
        WiT=np.ascontiguousarray(Wi_bond.T.astype(BF)),       # [DV+DE, DH]
        WfT=np.ascontiguousarray(Wf_bond.T.astype(BF)),       # [DH, DH]
        CT=np.ascontiguousarray(C.T.astype(BF)),
        CnT=np.ascontiguousarray((-C.T).astype(BF)),
        WoTv=np.ascontiguousarray(Wo_atom.T[:DV].astype(BF)),  # [DV, DH]
        WoTs=np.ascontiguousarray(Wo_atom.T[DV:].astype(BF)),  # [DH, DH]
        identb=np.eye(P, dtype=BF),
    )


# ---------------------------------------------------------------- bass program
def _build(meta, DEPTH_ITERS=2):
    import concourse.bass as bass
    import concourse.bacc as bacc
    import concourse.tile as tile
    from concourse import mybir

    F32 = mybir.dt.float32
    BF16 = mybir.dt.bfloat16
    I16 = mybir.dt.int16
    KP, APAD, DV, DE = meta["KP"], meta["APAD"], meta["DV"], meta["DE"]
    BA, BB, NR = meta["BA"], meta["BB"], meta["NR"]
    starts, n_r = meta["starts"], meta["n_r"]
    R_ROUNDS = len(n_r)
    DXT = DV + DE
    NFH = DH // P                      # feature halves (2)
    NA_CH = (APAD + CHUNK - 1) // CHUNK
    NT = KP // CHUNK
    NTA = meta["NTA"]                  # chunks in exchange region A (even)
    NW = CHUNK // 16                   # idx columns per chunk (32)
    SCB = 1                            # chunks per scatter call
    GB = 1                             # chunks per gather call

    nc = bacc.Bacc("TRN2", target_bir_lowering=False, debug=False, num_devices=NCORES,
                   num_swdge_queues=4)

    x_in = nc.dram_tensor("x", [DXT, KP], BF16, kind="ExternalInput")
    vfm_in = nc.dram_tensor("vfm", [DV, APAD], BF16, kind="ExternalInput")
    scidx_in = nc.dram_tensor("scidx", [P, NT * NW], I16, kind="ExternalInput")
    giidx_in = nc.dram_tensor("giidx", [P, NT * NW], I16, kind="ExternalInput")
    wiT_in = nc.dram_tensor("wiT", [DXT, DH], BF16, kind="ExternalInput")
    wfT_in = nc.dram_tensor("wfT", [DH, DH], BF16, kind="ExternalInput")
    ct_in = nc.dram_tensor("cT", [DH, DH], BF16, kind="ExternalInput")
    cnT_in = nc.dram_tensor("cnT", [DH, DH], BF16, kind="ExternalInput")
    woTv_in = nc.dram_tensor("woTv", [DV, DH], BF16, kind="ExternalInput")
    woTs_in = nc.dram_tensor("woTs", [DH, DH], BF16, kind="ExternalInput")
    idb_in = nc.dram_tensor("identb", [P, P], BF16, kind="ExternalInput")
    out_ext = nc.dram_tensor("out", [DH, APAD], BF16, kind="ExternalOutput")

    voutb = nc.dram_tensor("voutb", [DH, APAD], BF16)
    ccin = [nc.dram_tensor(f"ccin{i}", [NR, DH], BF16) for i in range(DEPTH_ITERS)]
    ccout = [nc.dram_tensor(f"ccout{i}", [NR, DH], BF16) for i in range(DEPTH_ITERS)]

    with tile.TileContext(nc) as tc:
        with (
            tc.tile_pool(name="wpool", bufs=1) as wp,
            tc.tile_pool(name="state", bufs=1) as st,
            tc.tile_pool(name="hin", bufs=2) as hp,
            tc.tile_pool(name="rout", bufs=2) as rp,
            tc.tile_pool(name="gpool", bufs=2) as gp,
            tc.tile_pool(name="psum", bufs=3, space="PSUM") as ps,
        ):
            # ---- resident weights
            wiT0 = wp.tile([P, DH], BF16, name="wiT0")
            nc.sync.dma_start(out=wiT0[:], in_=wiT_in[:P, :])
            wiT1 = wp.tile([DXT - P, DH], BF16, name="wiT1")
            nc.sync.dma_start(out=wiT1[:], in_=wiT_in[P:DXT, :])
            wfT = [wp.tile([P, DH], BF16, tag=f"wf{k}", name=f"wf{k}") for k in range(NFH)]
            cTb = [wp.tile([P, DH], BF16, tag=f"ct{k}", name=f"ct{k}") for k in range(NFH)]
            cnT = [wp.tile([P, DH], BF16, tag=f"cn{k}", name=f"cn{k}") for k in range(NFH)]
            for k in range(NFH):
                nc.sync.dma_start(out=wfT[k][:], in_=wfT_in[k * P:(k + 1) * P, :])
                nc.sync.dma_start(out=cTb[k][:], in_=ct_in[k * P:(k + 1) * P, :])
                nc.sync.dma_start(out=cnT[k][:], in_=cnT_in[k * P:(k + 1) * P, :])
            woTv0 = wp.tile([P, DH], BF16, name="woTv0")
            nc.sync.dma_start(out=woTv0[:], in_=woTv_in[:P, :])
            woTv1 = wp.tile([DV - P, DH], BF16, name="woTv1")
            nc.sync.dma_start(out=woTv1[:], in_=woTv_in[P:DV, :])
            woTs = [wp.tile([P, DH], BF16, tag=f"wo{k}", name=f"wo{k}") for k in range(NFH)]
            for k in range(NFH):
                nc.sync.dma_start(out=woTs[k][:], in_=woTs_in[k * P:(k + 1) * P, :])
            identb = wp.tile([P, P], BF16, name="identb")
            nc.sync.dma_start(out=identb[:], in_=idb_in[:, :])
            scidx = wp.tile([P, NT * NW], I16, name="scidx")
            nc.sync.dma_start(out=scidx[:], in_=scidx_in[:, :])
            giidx = wp.tile([P, NT * NW], I16, name="giidx")
            nc.sync.dma_start(out=giidx[:], in_=giidx_in[:, :])

            # ---- persistent SBUF state
            hL = [st.tile([P, KP], BF16, tag=f"hL{f}", name=f"hL{f}") for f in range(NFH)]
            asum = [st.tile([P, APAD], BF16, tag=f"as{f}", name=f"as{f}") for f in range(NFH)]
            asum2 = [st.tile([P, APAD], BF16, tag=f"as2{f}", name=f"as2{f}") for f in range(NFH)]

            def red_segments(c0):
                c1 = c0 + CHUNK
                segs = []
                for r in range(R_ROUNDS):
                    a = max(c0, int(starts[r]))
                    b = min(c1, int(starts[r + 1]))
                    if a >= b:
                        continue
                    segs.append((r, a, b, a - int(starts[r])))
                return segs

            def fused_accum(dst, c0):
                # accumulate freshly written hL[:, c0:c0+CHUNK] into dst by round
                for r, a, b, d0 in red_segments(c0):
                    for f in range(NFH):
                        if r == 0:
                            nc.vector.tensor_copy(out=dst[f][:, d0:d0 + (b - a)],
                                                  in_=hL[f][:, a:b])
                        else:
                            nc.vector.tensor_add(out=dst[f][:, d0:d0 + (b - a)],
                                                 in0=dst[f][:, d0:d0 + (b - a)],
                                                 in1=hL[f][:, a:b])

            def zero_tail(dst):
                n1 = int(n_r[0]) if R_ROUNDS else 0
                for f in range(NFH):
                    nc.vector.memset(dst[f][:, n1:APAD], 0)

            # segments of [c0, c0+CHUNK) by round; last round extends over the
            # pad tail (tail values go to the trash slot anyway)
            def segments(c0):
                c1 = c0 + CHUNK
                segs = []
                for r in range(R_ROUNDS):
                    a = max(c0, int(starts[r]))
                    b = c1 if r == R_ROUNDS - 1 else min(c1, int(starts[r + 1]))
                    if a >= b:
                        continue
                    segs.append((a, b, a - int(starts[r])))
                return segs

            # ---- init: H0 = relu(WiT.T @ X) into resident hL, asum fused
            def init_l():
                for t2 in range((NT + 1) // 2):
                    c0 = t2 * 2 * CHUNK
                    wdt = min(2 * CHUNK, KP - c0)
                    x0 = hp.tile([P, 2 * CHUNK], BF16, tag="x0", name="x0")
                    x1 = hp.tile([DXT - P, 2 * CHUNK], BF16, tag="x1", name="x1")
                    nc.sync.dma_start(out=x0[:, :wdt], in_=x_in[:P, c0:c0 + wdt])
                    nc.sync.dma_start(out=x1[:, :wdt], in_=x_in[P:DXT, c0:c0 + wdt])
                    for half_t in range(wdt // CHUNK):
                        t = t2 * 2 + half_t
                        xo = half_t * CHUNK
                        acc = ps.tile([P, NFH * CHUNK], F32, space="PSUM", tag="acc", name="acc")
                        for f in range(NFH):
                            o = f * CHUNK
                            nc.tensor.matmul(out=acc[:, o:o + CHUNK], lhsT=wiT0[:, f * P:(f + 1) * P],
                                             rhs=x0[:, xo:xo + CHUNK], start=True, stop=False)
                            nc.tensor.matmul(out=acc[:, o:o + CHUNK],
                                             lhsT=wiT1[:, f * P:(f + 1) * P], rhs=x1[:, xo:xo + CHUNK],
                                             start=False, stop=True)
                        for f in range(NFH):
                            nc.scalar.activation(out=hL[f][:, t * CHUNK:(t + 1) * CHUNK],
                                                 in_=acc[:, f * CHUNK:(f + 1) * CHUNK],
                                                 func=mybir.ActivationFunctionType.Relu)
                        fused_accum(asum, t * CHUNK)

            # ---- A4 = C.asum computed in place over asrc (bf16)
            def a4_compute(asrc):
                for t in range(NA_CH):
                    c0 = t * CHUNK
                    c1 = min(c0 + CHUNK, APAD)
                    n = c1 - c0
                    acc = ps.tile([P, NFH * CHUNK], F32, space="PSUM", tag="acc", name="acc")
                    for f in range(NFH):
                        o = f * CHUNK
                        for k in range(NFH):
                            nc.tensor.matmul(out=acc[:, o:o + n],
                                             lhsT=cTb[k][:, f * P:(f + 1) * P],
                                             rhs=asrc[k][:, c0:c1],
                                             start=(k == 0), stop=(k == NFH - 1))
                    for f in range(NFH):
                        nc.vector.tensor_copy(out=asrc[f][:, c0:c1],
                                              in_=acc[:, f * CHUNK:f * CHUNK + n])

            # ---- Vout staging: voutb = WoV.T @ Vfm over chunk range [t0, t1)
            def vout_stage(t0, t1):
                for t in range(t0, t1):
                    c0v = t * CHUNK
                    c1v = min(c0v + CHUNK, APAD)
                    n = c1v - c0v
                    vt0 = hp.tile([P, 2 * CHUNK], BF16, tag="x0", name="xv0")
                    vt1 = hp.tile([DXT - P, 2 * CHUNK], BF16, tag="x1", name="xv1")
                    nc.sync.dma_start(out=vt0[:, :n], in_=vfm_in[:P, c0v:c1v])
                    nc.sync.dma_start(out=vt1[:DV - P, :n], in_=vfm_in[P:DV, c0v:c1v])
                    accv = ps.tile([P, NFH * CHUNK], F32, space="PSUM", tag="acc", name="acc")
                    for f in range(NFH):
                        o = f * CHUNK
                        nc.tensor.matmul(out=accv[:, o:o + n], lhsT=woTv0[:, f * P:(f + 1) * P],
                                         rhs=vt0[:, :n], start=True, stop=False)
                        nc.tensor.matmul(out=accv[:, o:o + n], lhsT=woTv1[:, f * P:(f + 1) * P],
                                         rhs=vt1[:DV - P, :n], start=False, stop=True)
                    for f in range(NFH):
                        vo = rp.tile([P, CHUNK], BF16, tag=f"rr{f}", name=f"vo{f}")
                        nc.scalar.copy(out=vo[:, :n], in_=accv[:, f * CHUNK:f * CHUNK + n])
                        nc.sync.dma_start(out=voutb[f * P:(f + 1) * P, c0v:c1v], in_=vo[:, :n])

            # ================ program ================
            zero_tail(asum)
            init_l()

            # zero the exchange buffers (scatter is +=); emitted after init so
            # the writes ride on otherwise-idle queues during init compute
            zt = wp.tile([P, 1024], BF16, name="zt")
            nc.vector.memset(zt[:], 0)
            ZROWS = 1024 * P // DH                       # rows per full zt DMA
            for i in range(DEPTH_ITERS):
                r = 0
                while r < NR:
                    n = min(ZROWS, NR - r)
                    nc.sync.dma_start(out=ccin[i][r:r + n, :],
                                      in_=zt[:, :n * DH // P])
                    r += n

            for it in range(DEPTH_ITERS):
                asrc = asum if it == 0 else asum2
                adst = asum2 if it == 0 else asum
                a4_compute(asrc)
                zero_tail(adst)

                # ---- T-phase: T = A4[dst] - C.h per column, shipped as rows
                for t in range(NT):
                    c0 = t * CHUNK
                    q = t % SCB                          # slot within scatter batch
                    accT = ps.tile([P, NFH * CHUNK], F32, space="PSUM", tag="acc", name="acc")
                    for f in range(NFH):
                        o = f * CHUNK
                        for k in range(NFH):
                            nc.tensor.matmul(out=accT[:, o:o + CHUNK],
                                             lhsT=cnT[k][:, f * P:(f + 1) * P],
                                             rhs=hL[k][:, c0:c0 + CHUNK],
                                             start=(k == 0), stop=(k == NFH - 1))
                    for f in range(NFH):
                        o = f * CHUNK
                        for a, b, d0 in segments(c0):
                            nc.vector.tensor_add(out=accT[:, o + a - c0:o + b - c0],
                                                 in0=accT[:, o + a - c0:o + b - c0],
                                                 in1=asrc[f][:, d0:d0 + (b - a)])
                    tT = gp.tile([P, NFH, CHUNK], BF16, tag="tT", name="tT")
                    for f in range(NFH):
                        nc.vector.tensor_copy(out=tT[:, f, :],
                                              in_=accT[:, f * CHUNK:(f + 1) * CHUNK])
                    if q == 0:
                        rT = rp.tile([P, SCB * CHUNK // P, DH], BF16, tag="rT", name="rT")
                    for b in range(CHUNK // P):
                        tp = ps.tile([P, DH], F32, space="PSUM", tag="tp", name="tp", bufs=2)
                        for f in range(NFH):
                            nc.tensor.matmul(out=tp[:, f * P:(f + 1) * P],
                                             lhsT=tT[:, f, b * P:(b + 1) * P],
                                             rhs=identb[:], start=True, stop=True)
                        nc.scalar.copy(out=rT[:, q * (CHUNK // P) + b, :], in_=tp[:])
                    if q == SCB - 1:
                        t0 = t - (SCB - 1)
                        if t < NTA:
                            out_ap = ccin[it][0:NCORES * BA, :]
                        else:
                            out_ap = ccin[it][NCORES * BA:NR, :]
                        nc.gpsimd.dma_scatter_add(
                            out_ap=out_ap, in_ap=rT[:, :, :],
                            idxs_ap=scidx[:, t0 * NW:(t + 1) * NW],
                            num_idxs=SCB * CHUNK, num_idxs_reg=SCB * CHUNK, elem_size=DH,
                            queue_num=(t // SCB) % 4,
                        )
                    if t == NTA - 1:
                        nc.gpsimd.collective_compute(
                            "AllToAll", mybir.AluOpType.bypass,
                            replica_groups=[list(range(NCORES))],
                            ins=[ccin[it][0:NCORES * BA, :]],
                            outs=[ccout[it][0:NCORES * BA, :]],
                        )
                nc.gpsimd.collective_compute(
                    "AllToAll", mybir.AluOpType.bypass,
                    replica_groups=[list(range(NCORES))],
                    ins=[ccin[it][NCORES * BA:NR, :]],
                    outs=[ccout[it][NCORES * BA:NR, :]],
                )

                # fill the collective window with independent Vout staging
                if it == 0:
                    vout_stage(0, NA_CH // 2)
                else:
                    vout_stage(NA_CH // 2, NA_CH)

                # ---- U-phase: hL += relu(Wf.hL + T)
                for t in range(NT):
                    c0 = t * CHUNK
                    gq = t % GB
                    if gq == 0:
                        gn = min(GB, NT - t)
                        g = gp.tile([P, NFH, gn * CHUNK], BF16, tag="gg", name="g_t")
                        nc.gpsimd.dma_gather(
                            out_ap=g[:, :, :], in_ap=ccout[it][:, :],
                            idxs_ap=giidx[:, t * NW:(t + gn) * NW],
                            num_idxs=gn * CHUNK, num_idxs_reg=gn * CHUNK, elem_size=DH,
                            transpose=True, queue_num=(t // GB) % 4,
                        )
                    accU = ps.tile([P, NFH * CHUNK], F32, space="PSUM", tag="acc", name="acc")
                    for f in range(NFH):
                        o = f * CHUNK
                        for k in range(NFH):
                            nc.tensor.matmul(out=accU[:, o:o + CHUNK],
                                             lhsT=wfT[k][:, f * P:(f + 1) * P],
                                             rhs=hL[k][:, c0:c0 + CHUNK],
                                             start=(k == 0), stop=(k == NFH - 1))
                    for f in range(NFH):
                        o = f * CHUNK
                        nc.vector.tensor_add(out=accU[:, o:o + CHUNK],
                                             in0=accU[:, o:o + CHUNK],
                                             in1=g[:, f, gq * CHUNK:(gq + 1) * CHUNK])
                    for f in range(NFH):
                        rl = rp.tile([P, CHUNK], BF16, tag=f"r{f}", name=f"r{f}")
                        nc.scalar.activation(out=rl[:], in_=accU[:, f * CHUNK:(f + 1) * CHUNK],
                                             func=mybir.ActivationFunctionType.Relu)
                        nc.vector.tensor_add(out=hL[f][:, c0:c0 + CHUNK],
                                             in0=hL[f][:, c0:c0 + CHUNK], in1=rl[:])
                    fused_accum(adst, c0)

            # ---- final: out = relu(Vout + WoS.T @ a_sum)
            for t in range(NA_CH):
                c0 = t * CHUNK
                c1 = min(c0 + CHUNK, APAD)
                n = c1 - c0
                vo = [rp.tile([P, CHUNK], BF16, tag=f"rr{f}", name=f"hr{f}") for f in range(NFH)]
                for f in range(NFH):
                    nc.sync.dma_start(out=vo[f][:, :n], in_=voutb[f * P:(f + 1) * P, c0:c1])
                acc = ps.tile([P, NFH * CHUNK], F32, space="PSUM", tag="acc", name="acc")
                for f in range(NFH):
                    o = f * CHUNK
                    for k in range(NFH):
                        nc.tensor.matmul(out=acc[:, o:o + n], lhsT=woTs[k][:, f * P:(f + 1) * P],
                                         rhs=asum[k][:, c0:c1], start=(k == 0), stop=(k == NFH - 1))
                for f in range(NFH):
                    o = f * CHUNK
                    nc.vector.tensor_add(out=acc[:, o:o + n], in0=acc[:, o:o + n], in1=vo[f][:, :n])
                for f in range(NFH):
                    ot = rp.tile([P, CHUNK], BF16, tag=f"r{f}", name=f"o{f}")
                    nc.scalar.activation(out=ot[:, :n], in_=acc[:, f * CHUNK:f * CHUNK + n],
                                         func=mybir.ActivationFunctionType.Relu)
                    nc.sync.dma_start(out=out_ext[f * P:(f + 1) * P, c0:c1], in_=ot[:, :n])

    nc.compile()
    return nc


# ---------------------------------------------------------------- entry point
def _run(inputs, trace=False):
    from concourse.bass_utils import run_bass_kernel_spmd

    V = np.asarray(inputs["V"], np.float32)
    E_feat = np.asarray(inputs["E_feat"], np.float32)
    meta = _preprocess(V, E_feat, np.asarray(inputs["edge_index"]),
                       np.asarray(inputs["rev_edge_index"]))
    wts = _weights(np.asarray(inputs["Wi_bond"], np.float32),
                   np.asarray(inputs["Wh_bond"], np.float32),
                   np.asarray(inputs["Wf_bond"], np.float32),
                   np.asarray(inputs["Wo_atom"], np.float32), meta["DV"])

    key = (meta["KP"], meta["APAD"], meta["BA"], meta["BB"],
           tuple(meta["starts"].tolist()))
    if key not in _cache:
        _cache[key] = _build(meta)
    nc = _cache[key]

    in_maps = []
    for c in range(NCORES):
        cd = meta["cores"][c]
        in_maps.append({
            "x": cd["X"], "vfm": cd["Vfm"], "scidx": cd["scidx"], "giidx": cd["giidx"],
            "wiT": wts["WiT"], "wfT": wts["WfT"], "cT": wts["CT"], "cnT": wts["CnT"],
            "woTv": wts["WoTv"], "woTs": wts["WoTs"], "identb": wts["identb"],
        })
    res = run_bass_kernel_spmd(nc, in_maps, core_ids=list(range(NCORES)), trace=trace)

    N, ASH = meta["N"], meta["ASH"]
    out = np.empty((N, DH), np.float32)
    for c in range(NCORES):
        o = np.asarray(res.results[c]["out"]).astype(np.float32)   # [DH, APAD]
        order = meta["cores"][c]["order"]
        out[c * ASH + order] = o[:, :ASH].T
    return out, res.exec_time_ns


def kernel(**inputs) -> np.ndarray:
    out, _ = _run(inputs, trace=False)
    return out
